# revision 9
# baseline (speedup 1.0000x reference)
"""Trainium2 Bass kernel for nn_Blur: depthwise 4x4 separable blur.

Reference semantics: upfirdn2d(x, k2, up=1, down=1, pad=(2,1,2,1)) with a
separable 4-tap kernel k2 = outer(k1, k1), k1 = [1,3,3,1]/4 (flip is a
no-op: the kernel is symmetric).  out[h,w] = sum_{i,j} k1[i] k1[j]
x[h-2+i, w-2+j] with zero padding.

Implementation: both 1-D passes are banded matmuls on the TensorEngine.
B[n, k] = k1[k-n+2] (zero outside the band / the array), so filtering
along an axis is out = B @ x along that axis.  Feeding the *data* as the
stationary operand (lhsT) makes each pass transpose its output:

  pass 1:  tT[w, n]   = sum_h  x[h, w]  * B.T[h, n]   (filter along H)
  pass 2:  out[h, w'] = sum_w  tT[w, h] * B.T[w, w']  (filter along W)

so two passes come back to the natural [h, w] layout with no explicit
transposes.  Matmuls run as float32r (full fp32 data path on the PE at
1 cycle/row for free dims >= 256).

Sharding: pure data parallel — batch dim (8) across the 8 cores.
"""

import numpy as np

import bass_rust
import concourse.bass as bass
import concourse.mybir as mybir
from concourse.tile import TileContext
from concourse.vector_clock import ScopedClock
from concourse.bass_utils import run_bass_kernel_spmd

N_CORES = 8
C, H, W = 256, 256, 256
P = 128
G = 8  # planes per DMA group (2 MiB per transfer)
PAD0 = 2
TAPS = 4
DT = mybir.dt.float32
DTR = mybir.dt.float32r


class _TileContextPatched(TileContext):
    """TileContext whose tail drain splits semaphore waits across
    single-wait nops: the bundled walrus rejects >1 sync wait per
    non-EventSemaphore instruction, while stock Tile piles every live
    semaphore's wait onto the one tail Drain."""

    def _drain_and_barrier(self, tick_clock, wait_clock):
        nc = self.nc
        probe = nc.sync.nop(nofuse=True)
        wait_clock.add_sem_waits(
            probe.ins, ScopedClock({None: tick_clock.global_clock})
        )
        si = probe.ins.sync_info
        waits = list(si.on_wait) if si is not None else []
        updates = list(si.on_update) if si is not None else []
        if len(waits) > 1:
            probe.ins.sync_info = bass_rust.SyncInfo(
                on_wait=waits[:1], on_update=updates
            )
            for w in waits[1:]:
                extra = nc.sync.nop(nofuse=True)
                extra.ins.sync_info = bass_rust.SyncInfo(on_wait=[w], on_update=[])
        nc.sync.drain()
        nc.all_engine_barrier()
        assert self.sems is not None
        popped = nc._tile_sem_poison_stack.pop()
        assert popped is self._sem_poison
        nc.clear_and_free_semaphores(list(self.sems.allocated().values()))
        nc.all_engine_barrier()


def _split_multi_waits(nc: bass.Bass) -> bass.Bass:
    """The bundled walrus accepts at most 1 sync wait per instruction (2
    for EventSemaphore).  Tile's wait assignment attaches up to ~3.  Hoist
    the surplus waits onto same-engine nops inserted right before the
    instruction — the engine is in-order, so semantics are unchanged."""
    ctr = 0
    for f in nc.m.functions:
        for b in f.blocks:
            out = []
            for inst in b.instructions:
                si = inst.sync_info
                limit = 2 if isinstance(inst, mybir.InstEventSemaphore) else 1
                if si is not None and len(si.on_wait) > limit:
                    waits = list(si.on_wait)
                    kept, hoist = waits[-limit:], waits[:-limit]
                    for w in hoist:
                        ctr += 1
                        nop = mybir.InstNoOp(
                            name=f"I-waitsplit-{ctr}", engine=inst.engine
                        )
                        nop.sync_info = bass_rust.SyncInfo(
                            on_wait=[w], on_update=[]
                        )
                        nc.register_instruction(nop)
                        out.append(nop)
                    inst.sync_info = bass_rust.SyncInfo(
                        on_wait=kept, on_update=list(si.on_update)
                    )
                out.append(inst)
            b.instructions[:] = out
    return nc


def _filter_bt(k2: np.ndarray, n: int) -> np.ndarray:
    """B.T for the 1-D pass: B[m, k] = k1[k - m + PAD0], zero-padded edges.

    k1 is recovered from the (separable, rank-1) 2-D kernel: k2 =
    outer(k1, k1), so k1 = k2[0, :] / sqrt(k2[0, 0])."""
    k2 = np.asarray(k2, np.float64)
    k1 = k2[0, :] / np.sqrt(k2[0, 0])
    B = np.zeros((n, n), np.float64)
    for m in range(n):
        for i in range(TAPS):
            k = m + i - PAD0
            if 0 <= k < n:
                B[m, k] = k1[i]
    return np.ascontiguousarray(B.T.astype(np.float32))


def round_to_f32r(a: np.ndarray) -> np.ndarray:
    """Round fp32 to the fp32r encoding: same IEEE-754 layout with the
    mantissa rounded (RNE) to 11 bits — low 12 bits zero."""
    u = np.ascontiguousarray(a, np.float32).view(np.uint32)
    lsb = (u >> np.uint32(12)) & np.uint32(1)
    u = (u + np.uint32(0x7FF) + lsb) & np.uint32(0xFFFFF000)
    return u.view(np.float32)


def build_nc(c_planes: int = C, g: int = G, mode: str = "f32r") -> bass.Bass:
    """One core's program: blur c_planes [H, W] planes independently."""
    assert c_planes % g == 0
    mdt = {"f32r": DTR, "f32": DT}[mode]
    nc = bass.Bass()
    x = nc.dram_tensor("x", [c_planes, H, W], mdt, kind="ExternalInput")
    bt = nc.dram_tensor("bt", [H, H], mdt, kind="ExternalInput")
    out = nc.dram_tensor("out", [c_planes, H, W], DT, kind="ExternalOutput")

    with _TileContextPatched(nc) as tc:
        with (
            tc.tile_pool(name="const", bufs=1) as cpool,
            tc.tile_pool(name="io", bufs=2) as iopool,
            tc.tile_pool(name="mid", bufs=3) as midpool,
            tc.tile_pool(name="ps", bufs=2, space="PSUM") as pspool,
        ):
            bts = []
            for k in range(2):
                t = cpool.tile([P, H], mdt, tag=f"bt{k}")
                nc.sync.dma_start(out=t[:, :], in_=bt[k * P : (k + 1) * P, :])
                bts.append(t)

            for gi in range(c_planes // g):
                xs = iopool.tile([P, g, 2, W], mdt, tag="x")
                nc.sync.dma_start(
                    out=xs[:, :, :, :],
                    in_=x[gi * g : (gi + 1) * g, :, :].rearrange(
                        "c (k p) w -> p (c k) w", p=P
                    ),
                )
                os = iopool.tile([P, g, 2, W], DT, tag="o")
                for ci in range(g):
                    # pass 1: tT[w, n] = sum_h x[h, w] * BT[h, n]
                    ps1 = pspool.tile([P, 2, H], DT, tag="ps1")
                    for m in range(2):
                        for k in range(2):
                            nc.tensor.matmul(
                                ps1[:, m, :],
                                lhsT=xs[:, ci, k, m * P : (m + 1) * P],
                                rhs=bts[k][:, :],
                                start=(k == 0),
                                stop=(k == 1),
                            )
                    tt = midpool.tile([P, 2, H], mdt, tag="tt")
                    nc.scalar.copy(out=tt[:, :, :], in_=ps1[:, :, :])
                    # pass 2: out[h, w'] = sum_w tT[w, h] * BT[w, w']
                    ps2 = pspool.tile([P, 2, W], DT, tag="ps2")
                    for m in range(2):
                        for k in range(2):
                            nc.tensor.matmul(
                                ps2[:, m, :],
                                lhsT=tt[:, k, m * P : (m + 1) * P],
                                rhs=bts[k][:, :],
                                start=(k == 0),
                                stop=(k == 1),
                            )
                    nc.vector.tensor_copy(out=os[:, ci, :, :], in_=ps2[:, :, :])
                nc.sync.dma_start(
                    out=out[gi * g : (gi + 1) * g, :, :].rearrange(
                        "c (m p) w -> p (c m) w", p=P
                    ),
                    in_=os[:, :, :, :],
                )
    return _split_multi_waits(nc)


def kernel(x: np.ndarray, kernel: np.ndarray) -> np.ndarray:
    x = np.asarray(x)
    in_dtype = x.dtype
    assert x.shape == (N_CORES, C, H, W), x.shape
    xf = round_to_f32r(np.ascontiguousarray(x, dtype=np.float32))
    btm = round_to_f32r(_filter_bt(np.asarray(kernel, np.float32), H))
    nc = build_nc()
    in_maps = [{"x": xf[i], "bt": btm} for i in range(N_CORES)]
    res = run_bass_kernel_spmd(nc, in_maps, list(range(N_CORES)))
    out = np.stack([res.results[i]["out"] for i in range(N_CORES)], axis=0)
    return out.astype(in_dtype, copy=False)


# revision 11
# speedup vs baseline: 20.2014x; 20.2014x over previous
"""Trainium2 Bass kernel for nn_Blur: depthwise 4x4 separable blur.

Reference semantics: upfirdn2d(x, k2, up=1, down=1, pad=(2,1,2,1)) with a
separable 4-tap kernel k2 = outer(k1, k1), k1 = [1,3,3,1]/4 (flip is a
no-op: the kernel is symmetric).  out[h,w] = sum_{i,j} k1[i] k1[j]
x[h-2+i, w-2+j] with zero padding.

Implementation: both 1-D passes are banded matmuls on the TensorEngine.
B[n, k] = k1[k-n+2] (zero outside the band / the array), so filtering
along an axis is out = B @ x along that axis.  Feeding the *data* as the
stationary operand (lhsT) makes each pass transpose its output:

  pass 1:  tT[w, n]   = sum_h  x[h, w]  * B.T[h, n]   (filter along H)
  pass 2:  out[h, w'] = sum_w  tT[w, h] * B.T[w, w']  (filter along W)

so two passes come back to the natural [h, w] layout with no explicit
transposes.  Matmuls run as float32r (full fp32 data path on the PE at
1 cycle/row for free dims >= 256).

Sharding: pure data parallel — batch dim (8) across the 8 cores.
"""

import numpy as np

import bass_rust
import concourse.bass as bass
import concourse.mybir as mybir
from concourse.tile import TileContext
from concourse.vector_clock import ScopedClock
from concourse.bass_utils import run_bass_kernel_spmd

N_CORES = 8
C, H, W = 256, 256, 256
P = 128
G = 8  # planes per DMA group (2 MiB per transfer)
PAD0 = 2
TAPS = 4
DT = mybir.dt.float32
DTR = mybir.dt.float32r


class _TileContextPatched(TileContext):
    """TileContext whose tail drain splits semaphore waits across
    single-wait nops: the bundled walrus rejects >1 sync wait per
    non-EventSemaphore instruction, while stock Tile piles every live
    semaphore's wait onto the one tail Drain."""

    def _drain_and_barrier(self, tick_clock, wait_clock):
        nc = self.nc
        probe = nc.sync.nop(nofuse=True)
        wait_clock.add_sem_waits(
            probe.ins, ScopedClock({None: tick_clock.global_clock})
        )
        si = probe.ins.sync_info
        waits = list(si.on_wait) if si is not None else []
        updates = list(si.on_update) if si is not None else []
        if len(waits) > 1:
            probe.ins.sync_info = bass_rust.SyncInfo(
                on_wait=waits[:1], on_update=updates
            )
            for w in waits[1:]:
                extra = nc.sync.nop(nofuse=True)
                extra.ins.sync_info = bass_rust.SyncInfo(on_wait=[w], on_update=[])
        nc.sync.drain()
        nc.all_engine_barrier()
        assert self.sems is not None
        popped = nc._tile_sem_poison_stack.pop()
        assert popped is self._sem_poison
        nc.clear_and_free_semaphores(list(self.sems.allocated().values()))
        nc.all_engine_barrier()


def _split_multi_waits(nc: bass.Bass) -> bass.Bass:
    """The bundled walrus accepts at most 1 sync wait per instruction (2
    for EventSemaphore).  Tile's wait assignment attaches up to ~3.  Hoist
    the surplus waits onto same-engine nops inserted right before the
    instruction — the engine is in-order, so semantics are unchanged."""
    ctr = 0
    for f in nc.m.functions:
        for b in f.blocks:
            out = []
            for inst in b.instructions:
                si = inst.sync_info
                limit = 2 if isinstance(inst, mybir.InstEventSemaphore) else 1
                if si is not None and len(si.on_wait) > limit:
                    waits = list(si.on_wait)
                    kept, hoist = waits[-limit:], waits[:-limit]
                    for w in hoist:
                        ctr += 1
                        nop = mybir.InstNoOp(
                            name=f"I-waitsplit-{ctr}", engine=inst.engine
                        )
                        nop.sync_info = bass_rust.SyncInfo(
                            on_wait=[w], on_update=[]
                        )
                        nc.register_instruction(nop)
                        out.append(nop)
                    inst.sync_info = bass_rust.SyncInfo(
                        on_wait=kept, on_update=list(si.on_update)
                    )
                out.append(inst)
            b.instructions[:] = out
    return nc


def _filter_bt(k2: np.ndarray, n: int) -> np.ndarray:
    """B.T for the 1-D pass: B[m, k] = k1[k - m + PAD0], zero-padded edges.

    k1 is recovered from the (separable, rank-1) 2-D kernel: k2 =
    outer(k1, k1), so k1 = k2[0, :] / sqrt(k2[0, 0])."""
    k2 = np.asarray(k2, np.float64)
    k1 = k2[0, :] / np.sqrt(k2[0, 0])
    B = np.zeros((n, n), np.float64)
    for m in range(n):
        for i in range(TAPS):
            k = m + i - PAD0
            if 0 <= k < n:
                B[m, k] = k1[i]
    return np.ascontiguousarray(B.T.astype(np.float32))


def round_to_f32r(a: np.ndarray) -> np.ndarray:
    """Round fp32 to the fp32r encoding: same IEEE-754 layout with the
    mantissa rounded (RNE) to 11 bits — low 12 bits zero."""
    u = np.ascontiguousarray(a, np.float32).view(np.uint32)
    lsb = (u >> np.uint32(12)) & np.uint32(1)
    u = (u + np.uint32(0x7FF) + lsb) & np.uint32(0xFFFFF000)
    return u.view(np.float32)


def build_nc(
    c_planes: int = C,
    g: int = G,
    mode: str = "f32r",
    repeat: int = 1,
    io_bufs: int = 3,
    mid_bufs: int = 8,
    ps_bufs: int = 4,
) -> bass.Bass:
    """One core's program: blur c_planes [H, W] planes independently.

    repeat > 1 re-runs the whole sweep (for slope-based device timing)."""
    assert c_planes % g == 0
    mdt = {"f32r": DTR, "f32": DT}[mode]
    nc = bass.Bass()
    x = nc.dram_tensor("x", [c_planes, H, W], mdt, kind="ExternalInput")
    bt = nc.dram_tensor("bt", [H, H], mdt, kind="ExternalInput")
    out = nc.dram_tensor("out", [c_planes, H, W], DT, kind="ExternalOutput")

    with _TileContextPatched(nc) as tc:
        with (
            tc.tile_pool(name="const", bufs=1) as cpool,
            tc.tile_pool(name="io", bufs=io_bufs) as iopool,
            tc.tile_pool(name="mid", bufs=mid_bufs) as midpool,
            tc.tile_pool(name="ps", bufs=ps_bufs, space="PSUM") as pspool,
        ):
            bts = []
            for k in range(2):
                t = cpool.tile([P, H], mdt, tag=f"bt{k}")
                nc.sync.dma_start(out=t[:, :], in_=bt[k * P : (k + 1) * P, :])
                bts.append(t)

            for gi in [i for _ in range(repeat) for i in range(c_planes // g)]:
                xs = iopool.tile([P, g, 2, W], mdt, tag="x")
                nc.sync.dma_start(
                    out=xs[:, :, :, :],
                    in_=x[gi * g : (gi + 1) * g, :, :].rearrange(
                        "c (k p) w -> p (c k) w", p=P
                    ),
                )
                os = iopool.tile([P, g, 2, W], DT, tag="o")
                for ci in range(g):
                    # pass 1: tT[w, n] = sum_h x[h, w] * BT[h, n]
                    ps1 = pspool.tile([P, 2, H], DT, tag="ps1")
                    for m in range(2):
                        for k in range(2):
                            nc.tensor.matmul(
                                ps1[:, m, :],
                                lhsT=xs[:, ci, k, m * P : (m + 1) * P],
                                rhs=bts[k][:, :],
                                start=(k == 0),
                                stop=(k == 1),
                            )
                    tt = midpool.tile([P, 2, H], mdt, tag="tt")
                    nc.scalar.copy(out=tt[:, :, :], in_=ps1[:, :, :])
                    # pass 2: out[h, w'] = sum_w tT[w, h] * BT[w, w']
                    ps2 = pspool.tile([P, 2, W], DT, tag="ps2")
                    for m in range(2):
                        for k in range(2):
                            nc.tensor.matmul(
                                ps2[:, m, :],
                                lhsT=tt[:, k, m * P : (m + 1) * P],
                                rhs=bts[k][:, :],
                                start=(k == 0),
                                stop=(k == 1),
                            )
                    nc.vector.tensor_copy(out=os[:, ci, :, :], in_=ps2[:, :, :])
                nc.sync.dma_start(
                    out=out[gi * g : (gi + 1) * g, :, :].rearrange(
                        "c (m p) w -> p (c m) w", p=P
                    ),
                    in_=os[:, :, :, :],
                )
    return _split_multi_waits(nc)


def kernel(x: np.ndarray, kernel: np.ndarray) -> np.ndarray:
    x = np.asarray(x)
    in_dtype = x.dtype
    assert x.shape == (N_CORES, C, H, W), x.shape
    xf = round_to_f32r(np.ascontiguousarray(x, dtype=np.float32))
    btm = round_to_f32r(_filter_bt(np.asarray(kernel, np.float32), H))
    nc = build_nc()
    in_maps = [{"x": xf[i], "bt": btm} for i in range(N_CORES)]
    res = run_bass_kernel_spmd(nc, in_maps, list(range(N_CORES)))
    out = np.stack([res.results[i]["out"] for i in range(N_CORES)], axis=0)
    return out.astype(in_dtype, copy=False)


# revision 12
# speedup vs baseline: 21.4043x; 1.0595x over previous
"""Trainium2 Bass kernel for nn_Blur: depthwise 4x4 separable blur.

Reference semantics: upfirdn2d(x, k2, up=1, down=1, pad=(2,1,2,1)) with a
separable 4-tap kernel k2 = outer(k1, k1), k1 = [1,3,3,1]/4 (flip is a
no-op: the kernel is symmetric).  out[h,w] = sum_{i,j} k1[i] k1[j]
x[h-2+i, w-2+j] with zero padding.

Implementation: both 1-D passes are banded matmuls on the TensorEngine.
B[n, k] = k1[k-n+2] (zero outside the band / the array), so filtering
along an axis is out = B @ x along that axis.  Feeding the *data* as the
stationary operand (lhsT) makes each pass transpose its output:

  pass 1:  tT[w, n]   = sum_h  x[h, w]  * B.T[h, n]   (filter along H)
  pass 2:  out[h, w'] = sum_w  tT[w, h] * B.T[w, w']  (filter along W)

so two passes come back to the natural [h, w] layout with no explicit
transposes.  Matmuls run as float32r (full fp32 data path on the PE at
1 cycle/row for free dims >= 256).

Sharding: pure data parallel — batch dim (8) across the 8 cores.
"""

import numpy as np

import bass_rust
import concourse.bass as bass
import concourse.mybir as mybir
from concourse.tile import TileContext
from concourse.vector_clock import ScopedClock
from concourse.bass_utils import run_bass_kernel_spmd

N_CORES = 8
C, H, W = 256, 256, 256
P = 128
G = 8  # planes per DMA group (2 MiB per transfer)
PAD0 = 2
TAPS = 4
DT = mybir.dt.float32
DTR = mybir.dt.float32r


class _TileContextPatched(TileContext):
    """TileContext whose tail drain splits semaphore waits across
    single-wait nops: the bundled walrus rejects >1 sync wait per
    non-EventSemaphore instruction, while stock Tile piles every live
    semaphore's wait onto the one tail Drain."""

    def _drain_and_barrier(self, tick_clock, wait_clock):
        nc = self.nc
        probe = nc.sync.nop(nofuse=True)
        wait_clock.add_sem_waits(
            probe.ins, ScopedClock({None: tick_clock.global_clock})
        )
        si = probe.ins.sync_info
        waits = list(si.on_wait) if si is not None else []
        updates = list(si.on_update) if si is not None else []
        if len(waits) > 1:
            probe.ins.sync_info = bass_rust.SyncInfo(
                on_wait=waits[:1], on_update=updates
            )
            for w in waits[1:]:
                extra = nc.sync.nop(nofuse=True)
                extra.ins.sync_info = bass_rust.SyncInfo(on_wait=[w], on_update=[])
        nc.sync.drain()
        nc.all_engine_barrier()
        assert self.sems is not None
        popped = nc._tile_sem_poison_stack.pop()
        assert popped is self._sem_poison
        nc.clear_and_free_semaphores(list(self.sems.allocated().values()))
        nc.all_engine_barrier()


def _split_multi_waits(nc: bass.Bass) -> bass.Bass:
    """The bundled walrus accepts at most 1 sync wait per instruction (2
    for EventSemaphore).  Tile's wait assignment attaches up to ~3.  Hoist
    the surplus waits onto same-engine nops inserted right before the
    instruction — the engine is in-order, so semantics are unchanged."""
    ctr = 0
    for f in nc.m.functions:
        for b in f.blocks:
            out = []
            for inst in b.instructions:
                si = inst.sync_info
                limit = 2 if isinstance(inst, mybir.InstEventSemaphore) else 1
                if si is not None and len(si.on_wait) > limit:
                    waits = list(si.on_wait)
                    kept, hoist = waits[-limit:], waits[:-limit]
                    for w in hoist:
                        ctr += 1
                        nop = mybir.InstNoOp(
                            name=f"I-waitsplit-{ctr}", engine=inst.engine
                        )
                        nop.sync_info = bass_rust.SyncInfo(
                            on_wait=[w], on_update=[]
                        )
                        nc.register_instruction(nop)
                        out.append(nop)
                    inst.sync_info = bass_rust.SyncInfo(
                        on_wait=kept, on_update=list(si.on_update)
                    )
                out.append(inst)
            b.instructions[:] = out
    return nc


def _filter_bt(k2: np.ndarray, n: int) -> np.ndarray:
    """B.T for the 1-D pass: B[m, k] = k1[k - m + PAD0], zero-padded edges.

    k1 is recovered from the (separable, rank-1) 2-D kernel: k2 =
    outer(k1, k1), so k1 = k2[0, :] / sqrt(k2[0, 0])."""
    k2 = np.asarray(k2, np.float64)
    k1 = k2[0, :] / np.sqrt(k2[0, 0])
    B = np.zeros((n, n), np.float64)
    for m in range(n):
        for i in range(TAPS):
            k = m + i - PAD0
            if 0 <= k < n:
                B[m, k] = k1[i]
    return np.ascontiguousarray(B.T.astype(np.float32))


def round_to_f32r(a: np.ndarray) -> np.ndarray:
    """Round fp32 to the fp32r encoding: same IEEE-754 layout with the
    mantissa rounded (RNE) to 11 bits — low 12 bits zero."""
    u = np.ascontiguousarray(a, np.float32).view(np.uint32)
    lsb = (u >> np.uint32(12)) & np.uint32(1)
    u = (u + np.uint32(0x7FF) + lsb) & np.uint32(0xFFFFF000)
    return u.view(np.float32)


def build_nc(
    c_planes: int = C,
    g: int = G,
    mode: str = "f32r",
    repeat: int = 1,
    io_bufs: int = 3,
    mid_bufs: int = 8,
    ps_bufs: int = 4,
) -> bass.Bass:
    """One core's program: blur c_planes [H, W] planes independently.

    repeat > 1 re-runs the whole sweep (for slope-based device timing)."""
    assert c_planes % g == 0
    mdt = {"f32r": DTR, "f32": DT}[mode]
    nc = bass.Bass()
    x = nc.dram_tensor("x", [c_planes, H, W], mdt, kind="ExternalInput")
    bt = nc.dram_tensor("bt", [H, H], mdt, kind="ExternalInput")
    out = nc.dram_tensor("out", [c_planes, H, W], DT, kind="ExternalOutput")

    with _TileContextPatched(nc) as tc:
        with (
            tc.tile_pool(name="const", bufs=1) as cpool,
            tc.tile_pool(name="io", bufs=io_bufs) as iopool,
            tc.tile_pool(name="mid", bufs=mid_bufs) as midpool,
            tc.tile_pool(name="ps", bufs=ps_bufs, space="PSUM") as pspool,
        ):
            bts = []
            for k in range(2):
                t = cpool.tile([P, H], mdt, tag=f"bt{k}")
                nc.sync.dma_start(out=t[:, :], in_=bt[k * P : (k + 1) * P, :])
                bts.append(t)

            for gi in [i for _ in range(repeat) for i in range(c_planes // g)]:
                xs = iopool.tile([P, g, 2, W], mdt, tag="x")
                nc.sync.dma_start(
                    out=xs[:, :, :, :],
                    in_=x[gi * g : (gi + 1) * g, :, :].rearrange(
                        "c (k p) w -> p (c k) w", p=P
                    ),
                )
                os = iopool.tile([P, g, 2, W], DT, tag="o")
                for ci in range(g):
                    # pass 1: tT[w, n] = sum_h x[h, w] * BT[h, n]
                    ps1 = pspool.tile([P, 2, H], DT, tag="ps1")
                    for m in range(2):
                        for k in range(2):
                            nc.tensor.matmul(
                                ps1[:, m, :],
                                lhsT=xs[:, ci, k, m * P : (m + 1) * P],
                                rhs=bts[k][:, :],
                                start=(k == 0),
                                stop=(k == 1),
                            )
                    tt = midpool.tile([P, 2, H], mdt, tag="tt")
                    nc.scalar.copy(out=tt[:, :, :], in_=ps1[:, :, :])
                    # pass 2: out[h, w'] = sum_w tT[w, h] * BT[w, w']
                    ps2 = pspool.tile([P, 2, W], DT, tag="ps2")
                    for m in range(2):
                        for k in range(2):
                            nc.tensor.matmul(
                                ps2[:, m, :],
                                lhsT=tt[:, k, m * P : (m + 1) * P],
                                rhs=bts[k][:, :],
                                start=(k == 0),
                                stop=(k == 1),
                            )
                    nc.vector.tensor_copy(out=os[:, ci, :, :], in_=ps2[:, :, :])
                nc.sync.dma_start(
                    out=out[gi * g : (gi + 1) * g, :, :].rearrange(
                        "c (m p) w -> p (c m) w", p=P
                    ),
                    in_=os[:, :, :, :],
                )
    return _split_multi_waits(nc)


def kernel(x: np.ndarray, kernel: np.ndarray) -> np.ndarray:
    x = np.asarray(x)
    in_dtype = x.dtype
    assert x.shape == (N_CORES, C, H, W), x.shape
    xf = round_to_f32r(np.ascontiguousarray(x, dtype=np.float32))
    btm = round_to_f32r(_filter_bt(np.asarray(kernel, np.float32), H))
    in_maps = [{"x": xf[i], "bt": btm} for i in range(N_CORES)]
    res = None
    for attempt in range(3):
        try:
            nc = build_nc()
            res = run_bass_kernel_spmd(nc, in_maps, list(range(N_CORES)))
            break
        except Exception:
            # transient NRT/device hiccups have been observed; rebuild + retry
            if attempt == 2:
                raise
    out = np.stack([res.results[i]["out"] for i in range(N_CORES)], axis=0)
    return out.astype(in_dtype, copy=False)


# revision 17
# speedup vs baseline: 23.1269x; 1.0805x over previous
"""Trainium2 Bass kernel for nn_Blur: depthwise 4x4 separable blur.

Reference semantics: upfirdn2d(x, k2, up=1, down=1, pad=(2,1,2,1)) with a
separable 4-tap kernel k2 = outer(k1, k1), k1 = [1,3,3,1]/4 (flip is a
no-op: the kernel is symmetric).  out[h,w] = sum_{i,j} k1[i] k1[j]
x[h-2+i, w-2+j] with zero padding.

Implementation: both 1-D passes are banded matmuls on the TensorEngine.
B[n, k] = k1[k-n+2] (zero outside the band / the array), so filtering
along an axis is out = B @ x along that axis.  Feeding the *data* as the
stationary operand (lhsT) makes each pass transpose its output:

  pass 1:  tT[w, n]   = sum_h  x[h, w]  * B.T[h, n]   (filter along H)
  pass 2:  out[h, w'] = sum_w  tT[w, h] * B.T[w, w']  (filter along W)

so two passes come back to the natural [h, w] layout with no explicit
transposes.  Matmuls run as float32r (full fp32 data path on the PE at
1 cycle/row for free dims >= 256).

Sharding: pure data parallel — batch dim (8) across the 8 cores.
"""

import numpy as np

import bass_rust
import concourse.bass as bass
import concourse.mybir as mybir
from concourse.tile import TileContext
from concourse.vector_clock import ScopedClock
from concourse.bass_utils import run_bass_kernel_spmd

N_CORES = 8
C, H, W = 256, 256, 256
P = 128
G = 8  # planes per DMA group (2 MiB per transfer)
PAD0 = 2
TAPS = 4
DT = mybir.dt.float32
DTR = mybir.dt.float32r


class _TileContextPatched(TileContext):
    """TileContext whose tail drain splits semaphore waits across
    single-wait nops: the bundled walrus rejects >1 sync wait per
    non-EventSemaphore instruction, while stock Tile piles every live
    semaphore's wait onto the one tail Drain."""

    def _drain_and_barrier(self, tick_clock, wait_clock):
        nc = self.nc
        probe = nc.sync.nop(nofuse=True)
        wait_clock.add_sem_waits(
            probe.ins, ScopedClock({None: tick_clock.global_clock})
        )
        si = probe.ins.sync_info
        waits = list(si.on_wait) if si is not None else []
        updates = list(si.on_update) if si is not None else []
        if len(waits) > 1:
            probe.ins.sync_info = bass_rust.SyncInfo(
                on_wait=waits[:1], on_update=updates
            )
            for w in waits[1:]:
                extra = nc.sync.nop(nofuse=True)
                extra.ins.sync_info = bass_rust.SyncInfo(on_wait=[w], on_update=[])
        nc.sync.drain()
        nc.all_engine_barrier()
        assert self.sems is not None
        popped = nc._tile_sem_poison_stack.pop()
        assert popped is self._sem_poison
        nc.clear_and_free_semaphores(list(self.sems.allocated().values()))
        nc.all_engine_barrier()


def _split_multi_waits(nc: bass.Bass) -> bass.Bass:
    """The bundled walrus accepts at most 1 sync wait per instruction (2
    for EventSemaphore).  Tile's wait assignment attaches up to ~3.  Hoist
    the surplus waits onto same-engine nops inserted right before the
    instruction — the engine is in-order, so semantics are unchanged."""
    ctr = 0
    for f in nc.m.functions:
        for b in f.blocks:
            out = []
            for inst in b.instructions:
                si = inst.sync_info
                limit = 2 if isinstance(inst, mybir.InstEventSemaphore) else 1
                if si is not None and len(si.on_wait) > limit:
                    waits = list(si.on_wait)
                    kept, hoist = waits[-limit:], waits[:-limit]
                    for w in hoist:
                        ctr += 1
                        nop = mybir.InstNoOp(
                            name=f"I-waitsplit-{ctr}", engine=inst.engine
                        )
                        nop.sync_info = bass_rust.SyncInfo(
                            on_wait=[w], on_update=[]
                        )
                        nc.register_instruction(nop)
                        out.append(nop)
                    inst.sync_info = bass_rust.SyncInfo(
                        on_wait=kept, on_update=list(si.on_update)
                    )
                out.append(inst)
            b.instructions[:] = out
    return nc


def _filter_bt(k2: np.ndarray, n: int) -> np.ndarray:
    """B.T for the 1-D pass: B[m, k] = k1[k - m + PAD0], zero-padded edges.

    k1 is recovered from the (separable, rank-1) 2-D kernel: k2 =
    outer(k1, k1), so k1 = k2[0, :] / sqrt(k2[0, 0])."""
    k2 = np.asarray(k2, np.float64)
    k1 = k2[0, :] / np.sqrt(k2[0, 0])
    B = np.zeros((n, n), np.float64)
    for m in range(n):
        for i in range(TAPS):
            k = m + i - PAD0
            if 0 <= k < n:
                B[m, k] = k1[i]
    return np.ascontiguousarray(B.T.astype(np.float32))


def round_to_f32r(a: np.ndarray) -> np.ndarray:
    """Round fp32 to the fp32r encoding: same IEEE-754 layout with the
    mantissa rounded (RNE) to 11 bits — low 12 bits zero."""
    u = np.ascontiguousarray(a, np.float32).view(np.uint32)
    lsb = (u >> np.uint32(12)) & np.uint32(1)
    u = (u + np.uint32(0x7FF) + lsb) & np.uint32(0xFFFFF000)
    return u.view(np.float32)


def build_nc(
    c_planes: int = C,
    g: int = G,
    mode: str = "f32r",
    repeat: int = 1,
    io_bufs: int = 3,
    mid_bufs: int = 8,
    ps_bufs: int = 4,
    tt_parity_copy: bool = False,
) -> bass.Bass:
    """One core's program: blur c_planes [H, W] planes independently.

    repeat > 1 re-runs the whole sweep (for slope-based device timing)."""
    assert c_planes % g == 0
    mdt = {"f32r": DTR, "f32": DT}[mode]
    nc = bass.Bass()
    x = nc.dram_tensor("x", [c_planes, H, W], mdt, kind="ExternalInput")
    # bt: natural row order (rows 0..255); btp: parity-permuted rows
    # (btp[r*128 + p] = BT[2p + r]) to match the h-pair-per-partition
    # input layout below.
    bt = nc.dram_tensor("bt", [H, H], mdt, kind="ExternalInput")
    btp = nc.dram_tensor("btp", [H, H], mdt, kind="ExternalInput")
    out = nc.dram_tensor("out", [c_planes, H, W], DT, kind="ExternalOutput")

    with _TileContextPatched(nc) as tc:
        with (
            tc.tile_pool(name="const", bufs=1) as cpool,
            tc.tile_pool(name="io", bufs=io_bufs) as iopool,
            tc.tile_pool(name="mid", bufs=mid_bufs) as midpool,
            tc.tile_pool(name="ps", bufs=ps_bufs, space="PSUM") as pspool,
        ):
            bts, btps = [], []
            for k in range(2):
                t = cpool.tile([P, H], mdt, tag=f"bt{k}")
                nc.sync.dma_start(out=t[:, :], in_=bt[k * P : (k + 1) * P, :])
                bts.append(t)
                t = cpool.tile([P, H], mdt, tag=f"btp{k}")
                nc.sync.dma_start(out=t[:, :], in_=btp[k * P : (k + 1) * P, :])
                btps.append(t)

            for gi in [i for _ in range(repeat) for i in range(c_planes // g)]:
                # partition p holds the h-pair {2p, 2p+1}: 2 KiB HBM runs
                xs = iopool.tile([P, g, 2, W], mdt, tag="x")
                nc.sync.dma_start(
                    out=xs[:, :, :, :],
                    in_=x[gi * g : (gi + 1) * g, :, :].rearrange(
                        "c (p r) w -> p c r w", p=P
                    ),
                )
                os = iopool.tile([P, g, 2, W], DT, tag="o")
                for ci in range(g):
                    # pass 1: tT[w, n] = sum_h x[h, w] * BT[h, n]
                    # contraction over h split by parity r (h = 2p + r)
                    ps1 = pspool.tile([P, 2, H], DT, tag="ps1")
                    for m in range(2):
                        for r in range(2):
                            nc.tensor.matmul(
                                ps1[:, m, :],
                                lhsT=xs[:, ci, r, m * P : (m + 1) * P],
                                rhs=btps[r][:, :],
                                start=(r == 0),
                                stop=(r == 1),
                            )
                    # pass 2: out[h, w'] = sum_w tT[w, h] * BT[w, w']
                    # M (output h) sliced by parity so partition p of ps2
                    # holds h = 2p + t, matching the output DMA layout.
                    # tt_parity_copy: do the parity deinterleave during the
                    # PSUM evacuation (strided read) so pass-2 weight loads
                    # stay contiguous; else slice tt with stride 2.
                    if tt_parity_copy:
                        tt = midpool.tile([P, 2, 2, P], mdt, tag="tt")
                        nc.scalar.copy(
                            out=tt[:, :, :, :],
                            in_=ps1[:, :, :].rearrange(
                                "p m (h r) -> p m r h", r=2
                            ),
                        )
                        ttp = tt
                    else:
                        tt = midpool.tile([P, 2, H], mdt, tag="tt")
                        nc.scalar.copy(out=tt[:, :, :], in_=ps1[:, :, :])
                        ttp = tt[:, :, :].rearrange("p m (h r) -> p m r h", r=2)
                    ps2 = pspool.tile([P, 2, W], DT, tag="ps2")
                    for t in range(2):
                        for k in range(2):
                            nc.tensor.matmul(
                                ps2[:, t, :],
                                lhsT=ttp[:, k, t, :],
                                rhs=bts[k][:, :],
                                start=(k == 0),
                                stop=(k == 1),
                            )
                    nc.vector.tensor_copy(out=os[:, ci, :, :], in_=ps2[:, :, :])
                nc.sync.dma_start(
                    out=out[gi * g : (gi + 1) * g, :, :].rearrange(
                        "c (p r) w -> p c r w", p=P
                    ),
                    in_=os[:, :, :, :],
                )
    return _split_multi_waits(nc)


def parity_permute_rows(btm: np.ndarray) -> np.ndarray:
    """btp[r*128 + p] = btm[2p + r]."""
    n = btm.shape[0]
    return np.ascontiguousarray(
        btm.reshape(n // 2, 2, n).transpose(1, 0, 2).reshape(n, n)
    )


def kernel(x: np.ndarray, kernel: np.ndarray) -> np.ndarray:
    x = np.asarray(x)
    in_dtype = x.dtype
    assert x.shape == (N_CORES, C, H, W), x.shape
    xf = round_to_f32r(np.ascontiguousarray(x, dtype=np.float32))
    btm = round_to_f32r(_filter_bt(np.asarray(kernel, np.float32), H))
    btpm = parity_permute_rows(btm)
    in_maps = [{"x": xf[i], "bt": btm, "btp": btpm} for i in range(N_CORES)]
    res = None
    for attempt in range(3):
        try:
            nc = build_nc()
            res = run_bass_kernel_spmd(nc, in_maps, list(range(N_CORES)))
            break
        except Exception:
            # transient NRT/device hiccups have been observed; rebuild + retry
            if attempt == 2:
                raise
    out = np.stack([res.results[i]["out"] for i in range(N_CORES)], axis=0)
    return out.astype(in_dtype, copy=False)
